# revision 1
# baseline (speedup 1.0000x reference)
"""Multi-head self-attention (B=2, S=2048, D=1024, H=16) on 8 Trainium2 NeuronCores.

Sharding: batch x head-group. Core c = b*4 + g handles batch b and heads 4g..4g+3
(Megatron-style TP: Wq/Wk/Wv column-sharded, Wo row-sharded; partial outputs
summed on the host).

Per-core kernel layout ("T-layout": sequence on the free dim everywhere):
  inputs (host-prepared):  xt [1024, 2048] = x[b].T;  wq/wk/wv [1024, 256]
  (scale-folded, transposed);  wo [256, 1024] (scale-folded, transposed)
  QT/KT = (w.T @ xt) [256, 2048]        d' on partitions, heads pair-stacked
  V     = (xt.T @ wv) [2048, 260]       natural layout + ones column per head
  scoresT[k, q] = KT_h-slices.T @ QT_h  per head, k on partitions
  expT = exp(scoresT / 8)               (no max subtraction: |scores| <~ 2)
  ctxT_aug[d+1, q] = [V_h | 1].T @ expT accumulated over k-chunks; row 64 = denom
  ctxT = ctxT_aug[0:64] * (1/denom)     denominator broadcast via gpsimd
  outT_partial = wo.T @ ctxT [1024, 2048]
Host: out[b] = sum_g outT[b, g].T

Every matmul uses K<=64 contraction (row-tiled 64x128 PE mode, tiles T0/T8
run concurrently) so the PE never switches tiling modes.
"""
import sys

sys.path.insert(0, "/opt/trn_rl_repo")

import numpy as np

import concourse.bass as bass
import concourse.tile as tile
from concourse import bacc, mybir
from concourse.bass_utils import run_bass_kernel_spmd

F32 = mybir.dt.float32
MM_DT = mybir.dt.float32r  # 1 cycle/row at N>=256 (fp32 is 4); fp32 storage

S = 2048          # sequence length per batch
D = 1024          # embedding dim
HG = 4            # heads per core
HD = 64           # head dim
GC = HG * HD      # group cols = 256
P = 128
NQ = 4            # q chunks of 512
QW = 512          # q chunk width
NKC = 16          # key-position chunks of 128
KO = 8            # contraction chunks of 128 over D
VW = HD + 1       # V columns per head incl. ones column

_NC_CACHE = {}
DEBUG_DUMPS = False


def _build():
    if "nc" in _NC_CACHE:
        return _NC_CACHE["nc"]
    nc = bacc.Bacc(trn_type="TRN2", target_bir_lowering=False, debug=False)
    xt_d = nc.dram_tensor("xt", [D, S], MM_DT, kind="ExternalInput")
    wq_d = nc.dram_tensor("wq", [D, GC], MM_DT, kind="ExternalInput")
    wk_d = nc.dram_tensor("wk", [D, GC], MM_DT, kind="ExternalInput")
    wv_d = nc.dram_tensor("wv", [D, GC], MM_DT, kind="ExternalInput")
    wo_d = nc.dram_tensor("wo", [GC, D], MM_DT, kind="ExternalInput")
    out_d = nc.dram_tensor("out_t", [D, S], F32, kind="ExternalOutput")
    dbg = None
    if DEBUG_DUMPS:
        dbg = {
            "dbg_qt": nc.dram_tensor("dbg_qt", [P, 2, S], MM_DT, kind="ExternalOutput"),
            "dbg_kt": nc.dram_tensor("dbg_kt", [P, 2, S], MM_DT, kind="ExternalOutput"),
            "dbg_va": nc.dram_tensor("dbg_va", [P, NKC, HG * VW], MM_DT,
                                     kind="ExternalOutput"),
            "dbg_ct": nc.dram_tensor("dbg_ct", [P, 2, S], MM_DT, kind="ExternalOutput"),
            "dbg_ex": nc.dram_tensor("dbg_ex", [P, 4, QW], MM_DT, kind="ExternalOutput"),
            "dbg_sc": nc.dram_tensor("dbg_sc", [P, 4, QW], F32, kind="ExternalOutput"),
        }

    scr_d = nc.dram_tensor("nrm_scratch", [2, NQ, 2, QW], F32)
    with tile.TileContext(nc) as tc:
        _emit(nc, tc, xt_d, wq_d, wk_d, wv_d, wo_d, out_d, scr_d, dbg)
    nc.compile()
    _NC_CACHE["nc"] = nc
    return nc


def _emit(nc, tc, xt_d, wq_d, wk_d, wv_d, wo_d, out_d, scr_d, dbg=None):
    with tc.tile_pool(name="big", bufs=1) as big:
        # ---- persistent SBUF tensors (~96KB/partition) ----
        wo_sb = big.tile([P, 2, D], MM_DT)        # [d'(128) x chunk x e]
        qt = big.tile([P, 2, S], MM_DT)           # QT: head h at parts (h%2)*64, chunk h//2
        kt = big.tile([P, 2, S], MM_DT)
        va = big.tile([P, NKC, HG * VW], MM_DT)   # V natural + ones col per head
        ct = big.tile([P, 2, S], MM_DT)           # ctxT, same head layout as qt

        nc.sync.dma_start(wo_sb[:], wo_d.rearrange("(c p) e -> p c e", p=P))

        # ones columns of V_aug (col HD of each VW-wide head block)
        va_h = va[:].rearrange("p s (h c) -> p s h c", c=VW)
        for h in range(HG):
            # fp32 1.0 bit pattern; walrus memset rejects float32r directly
            nc.vector.memset(
                va_h[:, :, h, HD:HD + 1].bitcast(mybir.dt.uint32), 0x3F800000)

        def mm_pair(pa, pb, lhsT, rhs, start, stop):
            """Row-tiled K=64 pair: T0 (parts 0-63) -> pa, T8 (parts 64-127) -> pb."""
            nc.tensor.matmul(pa, lhsT[0:64], rhs[0:64], start=start, stop=stop)
            nc.tensor.matmul(pb, lhsT[64:128], rhs[64:128], start=start, stop=stop)

        # ================= phase 1: projections =================
        with tc.tile_pool(name="xw", bufs=1) as xw, \
             tc.tile_pool(name="evac", bufs=3) as evac, \
             tc.tile_pool(name="ps_proj", bufs=4, space="PSUM") as ps_proj:
            xs = xw.tile([P, KO, S], MM_DT)       # x.T, [d_in(128) x ko x s]
            wq = xw.tile([P, KO, GC], MM_DT)
            wk = xw.tile([P, KO, GC], MM_DT)
            wv = xw.tile([P, KO, GC], MM_DT)
            for ko in range(KO):
                nc.sync.dma_start(xs[:, ko, :], xt_d[ko * P:(ko + 1) * P, :])
            nc.sync.dma_start(wq[:], wq_d.rearrange("(ko p) m -> p ko m", p=P))
            nc.sync.dma_start(wk[:], wk_d.rearrange("(ko p) m -> p ko m", p=P))
            nc.sync.dma_start(wv[:], wv_d.rearrange("(ko p) m -> p ko m", p=P))

            # QT/KT: transposed out [d' x s]
            for w_sb, dst in ((wq, qt), (wk, kt)):
                for m in range(2):          # d' chunk = head pair
                    for n in range(NQ):
                        pa = ps_proj.tile([P, QW], F32, tag="pp")
                        pb = ps_proj.tile([P, QW], F32, tag="pp")
                        for ko in range(KO):
                            mm_pair(pa[:], pb[:],
                                    w_sb[:, ko, m * P:(m + 1) * P],
                                    xs[:, ko, n * QW:(n + 1) * QW],
                                    start=(ko == 0), stop=(ko == KO - 1))
                        t = evac.tile([P, QW], F32, tag="ev")
                        nc.vector.tensor_copy(t[:], pb[:])
                        nc.vector.tensor_tensor(
                            dst[:, m, n * QW:(n + 1) * QW],
                            pa[:], t[:], mybir.AluOpType.add)

            # V natural: [s(128) x 256] per s-chunk
            for sc in range(NKC):
                pa = ps_proj.tile([P, QW], F32, tag="pp")
                pb = ps_proj.tile([P, QW], F32, tag="pp")
                for ko in range(KO):
                    mm_pair(pa[:, :GC], pb[:, :GC],
                            xs[:, ko, sc * P:(sc + 1) * P],
                            wv[:, ko, :],
                            start=(ko == 0), stop=(ko == KO - 1))
                tv = evac.tile([P, QW], F32, tag="ev")
                nc.vector.tensor_copy(tv[:, :GC], pb[:, :GC])
                nc.vector.tensor_tensor(
                    va_h[:, sc, :, 0:HD],
                    pa[:, :GC].rearrange("p (h c) -> p h c", c=HD),
                    tv[:, :GC].rearrange("p (h c) -> p h c", c=HD),
                    mybir.AluOpType.add)

        if dbg is not None:
            nc.sync.dma_start(dbg["dbg_qt"][:], qt[:])
            nc.sync.dma_start(dbg["dbg_kt"][:], kt[:])
            nc.sync.dma_start(dbg["dbg_va"][:], va[:])

        # ================= phase 2: attention =================
        with tc.tile_pool(name="expp", bufs=2) as expp, \
             tc.tile_pool(name="norm", bufs=2) as norm, \
             tc.tile_pool(name="ps_sc", bufs=1, space="PSUM") as ps_sc, \
             tc.tile_pool(name="ps_ctx", bufs=1, space="PSUM") as ps_ctx:
            for hp in range(2):         # head pair (even = parts 0-63, odd = 64-127)
                for n in range(NQ):
                    # ctx accumulators: [even/odd head] x [k-low/k-high half]
                    cps = [[ps_ctx.tile([P, QW], F32, tag=f"pc{e}{l}",
                                        name=f"pc{e}{l}_{hp}_{n}")
                            for l in range(2)] for e in range(2)]
                    for kb in range(NKC // 2):
                        sp = ps_sc.tile([P, 4, QW], F32, tag="psc")   # 4 banks
                        ex = expp.tile([P, 4, QW], MM_DT, tag="pex")
                        for j in range(4):
                            kc = kb * 2 + j // 2
                            lo = (j % 2) * 64
                            nc.tensor.matmul(
                                sp[:, j, :],
                                kt[lo:lo + 64, hp, kc * P:(kc + 1) * P],
                                qt[lo:lo + 64, hp, n * QW:(n + 1) * QW],
                                start=True, stop=True)
                        nc.scalar.activation(
                            ex[:].rearrange("p a b -> p (a b)"),
                            sp[:].rearrange("p a b -> p (a b)"),
                            mybir.ActivationFunctionType.Exp,
                            scale=0.125)
                        if dbg is not None and hp == 0 and n == 0 and kb == 0:
                            nc.sync.dma_start(dbg["dbg_ex"][:], ex[:])
                            spc = norm.tile([P, 4, QW], F32, tag="spdump")
                            nc.vector.tensor_copy(spc[:], sp[:])
                            nc.sync.dma_start(dbg["dbg_sc"][:], spc[:])
                        for j in range(4):
                            kc = kb * 2 + j // 2
                            e = j % 2
                            h = 2 * hp + e
                            for l in range(2):   # k-low / k-high 64-halves
                                nc.tensor.matmul(
                                    cps[e][l][0:VW, :],
                                    va[l * 64:(l + 1) * 64, kc, h * VW:(h + 1) * VW],
                                    ex[l * 64:(l + 1) * 64, j, :],
                                    start=(kb == 0 and j < 2),
                                    stop=(kb == NKC // 2 - 1 and j >= 2))
                    # normalize: ctxT = (A+B)[0:64] / (A+B)[64]
                    for e in range(2):
                        sm = norm.tile([P, QW], F32, tag="nsum")
                        bc = norm.tile([P, QW], F32, tag="nbc")
                        nc.vector.tensor_copy(sm[0:VW, :], cps[e][1][0:VW, :])
                        nc.vector.tensor_tensor(sm[0:VW, :], cps[e][0][0:VW, :],
                                                sm[0:VW, :], mybir.AluOpType.add)
                        nc.vector.reciprocal(sm[HD:VW, :], sm[HD:VW, :])
                        # partition-broadcast 1/denom via DRAM bounce
                        sl = scr_d[hp, n, e]
                        nc.sync.dma_start(sl.unsqueeze(0), sm[HD:VW, :])
                        bc_src = bass.AP(tensor=sl.tensor, offset=sl.offset,
                                         ap=[[0, 64]] + list(sl.ap))
                        nc.sync.dma_start(bc[0:64, :], bc_src)
                        nc.vector.tensor_tensor(
                            ct[e * 64:e * 64 + 64, hp, n * QW:(n + 1) * QW],
                            sm[0:HD, :], bc[0:64, :], mybir.AluOpType.mult)

        if dbg is not None:
            nc.sync.dma_start(dbg["dbg_ct"][:], ct[:])

        # ================= phase 3: output projection =================
        with tc.tile_pool(name="outp", bufs=3) as outp, \
             tc.tile_pool(name="ps_o", bufs=4, space="PSUM") as ps_o:
            for m in range(KO):         # e chunks of 128
                for n in range(NQ):
                    pa = ps_o.tile([P, QW], F32, tag="po")
                    pb = ps_o.tile([P, QW], F32, tag="po")
                    for c in range(2):
                        mm_pair(pa[:], pb[:],
                                wo_sb[:, c, m * P:(m + 1) * P],
                                ct[:, c, n * QW:(n + 1) * QW],
                                start=(c == 0), stop=(c == 1))
                    ot = outp.tile([P, QW], F32, tag="ot")
                    nc.vector.tensor_copy(ot[:], pb[:])
                    nc.vector.tensor_tensor(ot[:], pa[:], ot[:],
                                            mybir.AluOpType.add)
                    nc.sync.dma_start(
                        out_d[m * P:(m + 1) * P, n * QW:(n + 1) * QW], ot[:])


def _in_maps(x, wq_f, wk_f, wv_f, wo_f):
    maps = []
    for core in range(8):
        b, g = core // 4, core % 4
        cols = slice(g * GC, (g + 1) * GC)
        maps.append({
            "xt": np.ascontiguousarray(x[b].T),
            "wq": np.ascontiguousarray(wq_f[:, cols]),
            "wk": np.ascontiguousarray(wk_f[:, cols]),
            "wv": np.ascontiguousarray(wv_f[:, cols]),
            "wo": np.ascontiguousarray(wo_f[cols, :]),
        })
    return maps


def run_traced(x, Wq, Wk, Wv, Wo, q_scale, k_scale, v_scale, o_scale):
    """Like kernel() but with NTFF tracing; returns (out, exec_time_ns, trace_path)."""
    x = np.asarray(x, dtype=np.float32)
    wq_f = (np.asarray(Wq).T * np.asarray(q_scale).reshape(1, -1)).astype(np.float32)
    wk_f = (np.asarray(Wk).T * np.asarray(k_scale).reshape(1, -1)).astype(np.float32)
    wv_f = (np.asarray(Wv).T * np.asarray(v_scale).reshape(1, -1)).astype(np.float32)
    wo_f = (np.asarray(Wo).T * np.asarray(o_scale).reshape(1, -1)).astype(np.float32)
    nc = _build()
    res = run_bass_kernel_spmd(nc, _in_maps(x, wq_f, wk_f, wv_f, wo_f),
                               core_ids=list(range(8)), trace=True)
    out = np.zeros((x.shape[0], S, D), dtype=np.float32)
    for core in range(8):
        out[core // 4] += res.results[core]["out_t"].T
    trace_path = None
    if res.instructions_and_trace is not None:
        trace_path = res.instructions_and_trace[1]
    return out, res.exec_time_ns, trace_path


def kernel(x, Wq, Wk, Wv, Wo, q_scale, k_scale, v_scale, o_scale):
    B = x.shape[0]
    x = np.asarray(x, dtype=np.float32)
    wq_f = (np.asarray(Wq).T * np.asarray(q_scale).reshape(1, -1)).astype(np.float32)
    wk_f = (np.asarray(Wk).T * np.asarray(k_scale).reshape(1, -1)).astype(np.float32)
    wv_f = (np.asarray(Wv).T * np.asarray(v_scale).reshape(1, -1)).astype(np.float32)
    wo_f = (np.asarray(Wo).T * np.asarray(o_scale).reshape(1, -1)).astype(np.float32)

    nc = _build()
    res = run_bass_kernel_spmd(nc, _in_maps(x, wq_f, wk_f, wv_f, wo_f),
                               core_ids=list(range(8)))
    out = np.zeros((B, S, D), dtype=np.float32)
    for core in range(8):
        b = core // 4
        out[b] += res.results[core]["out_t"].T
    return out



# revision 15
# speedup vs baseline: 1.6108x; 1.6108x over previous
"""Multi-head self-attention (B=2, S=2048, D=1024, H=16) on 8 Trainium2 NeuronCores.

Sharding: batch x head-group. Core c = b*4 + g handles batch b and heads 4g..4g+3
(Megatron-style TP: Wq/Wk/Wv column-sharded, Wo row-sharded; partial outputs
summed on the host).

Per-core kernel layout ("T-layout": sequence on the free dim everywhere),
all matmul operands bf16, PSUM accumulation fp32:
  inputs (host-prepared):  xt [1024, 2048] = x[b].T;  wq/wk/wv [1024, 256]
  (scale-folded, transposed);  wo [256, 1024] (scale-folded, transposed)
  QT/KT = (w.T @ xt) [256, 2048]        d' on partitions, heads pair-stacked
  V     = (xt.T @ wv) [2048, 260]       natural layout + ones column per head
  scoresT[k, q] = KT_h-slices.T @ QT_h  per head, k on partitions (row-tiled
                                        T0/T8 pair: both heads of a pair run
                                        concurrently on the PE)
  expT = exp(scoresT / 8)               (no max subtraction: |scores| <~ 2)
  ctxT_aug[d+1, q] = [V_h | 1].T @ expT K=128 accumulation in one PSUM bank;
                                        row 64 = softmax denominator
  ctxT = ctxT_aug[0:64] * (1/denom)     recip on DVE, denom row broadcast via
                                        gpsimd partition_broadcast
  outT_partial = wo.T @ ctxT [1024, 2048]
Host: out[b] = sum_g outT[b, g].T

Pipeline structure: phase-2 score PSUM double-buffered so the Exp ACTIVATEs
(the critical path, ~128 x [128,1024]) stream back-to-back on the Scalar
engine while the PE interleaves scores/ctx with "filler" work (V projection,
remaining QT tiles, per-n output projection) to stay HAM-warm.
"""
import sys

sys.path.insert(0, "/opt/trn_rl_repo")

import numpy as np
import ml_dtypes

import concourse.bass as bass
import concourse.tile as tile
from concourse import bacc, mybir
from concourse.bass_utils import run_bass_kernel_spmd

F32 = mybir.dt.float32
BF16 = mybir.dt.bfloat16
NP_BF16 = ml_dtypes.bfloat16

S = 2048          # sequence length per batch
D = 1024          # embedding dim
HG = 4            # heads per core
HD = 64           # head dim
GC = HG * HD      # group cols = 256
P = 128
NQ = 4            # q chunks of 512
QW = 512          # q chunk width
NKC = 16          # key-position chunks of 128
KO = 8            # contraction chunks of 128 over D
VW = HD + 1       # V columns per head incl. ones column

_NC_CACHE = {}
DEBUG_DUMPS = False


def _build():
    if "nc" in _NC_CACHE:
        return _NC_CACHE["nc"]
    nc = bacc.Bacc(trn_type="TRN2", target_bir_lowering=False, debug=False)
    xt_d = nc.dram_tensor("xt", [D, S], BF16, kind="ExternalInput")
    wq_d = nc.dram_tensor("wq", [D, GC], BF16, kind="ExternalInput")
    wk_d = nc.dram_tensor("wk", [D, GC], BF16, kind="ExternalInput")
    wv_d = nc.dram_tensor("wv", [D, GC], BF16, kind="ExternalInput")
    wo_d = nc.dram_tensor("wo", [GC, D], BF16, kind="ExternalInput")
    out_d = nc.dram_tensor("out_t", [D, S], F32, kind="ExternalOutput")
    dbg = None
    if DEBUG_DUMPS:
        dbg = {
            "dbg_qt": nc.dram_tensor("dbg_qt", [P, 2, S], BF16,
                                     kind="ExternalOutput"),
            "dbg_kt": nc.dram_tensor("dbg_kt", [P, 2, S], BF16,
                                     kind="ExternalOutput"),
            "dbg_va": nc.dram_tensor("dbg_va", [P, NKC, HG * VW], BF16,
                                     kind="ExternalOutput"),
            "dbg_ct": nc.dram_tensor("dbg_ct", [P, 2, S], BF16,
                                     kind="ExternalOutput"),
            "dbg_dn": nc.dram_tensor("dbg_dn", [P, 4, QW], F32,
                                     kind="ExternalOutput"),
            "dbg_bc": nc.dram_tensor("dbg_bc", [P, 2, QW], F32,
                                     kind="ExternalOutput"),
            "dbg_ex": nc.dram_tensor("dbg_ex", [P, 2, QW], BF16,
                                     kind="ExternalOutput"),
        }
    with tile.TileContext(nc) as tc:
        _emit(nc, tc, xt_d, wq_d, wk_d, wv_d, wo_d, out_d, dbg)
    nc.compile()
    _NC_CACHE["nc"] = nc
    return nc


def _emit(nc, tc, xt_d, wq_d, wk_d, wv_d, wo_d, out_d, dbg=None):
    mult = mybir.AluOpType.mult
    with tc.tile_pool(name="big", bufs=1) as big, \
         tc.tile_pool(name="ex", bufs=4) as ex_pool, \
         tc.tile_pool(name="dn", bufs=2) as dn_pool, \
         tc.tile_pool(name="bcn", bufs=2) as bc_pool, \
         tc.tile_pool(name="ot", bufs=2) as ot_pool, \
         tc.tile_pool(name="ps_g", bufs=2, space="PSUM") as ps_g, \
         tc.tile_pool(name="ps_s", bufs=2, space="PSUM") as ps_s, \
         tc.tile_pool(name="ps_c", bufs=1, space="PSUM") as ps_c:

        # ---- persistent SBUF tensors ----
        xs = big.tile([P, KO, S], BF16)          # x.T  [d_in(128) x ko x s]
        wqs = big.tile([P, KO, GC], BF16)
        wks = big.tile([P, KO, GC], BF16)
        wvs = big.tile([P, KO, GC], BF16)
        wo_sb = big.tile([P, 2, D], BF16)        # [d'(128) x chunk x e]
        qt = big.tile([P, 2, S], BF16)           # head h at parts (h%2)*64, chunk h//2
        kt = big.tile([P, 2, S], BF16)
        va = big.tile([P, NKC, HG * VW], BF16)   # V natural + ones col per head
        ct = big.tile([P, 2, S], BF16)           # normalized ctxT, same layout as qt

        # ---- input DMAs: weights first (small), then x by q-block so the
        # first KT tile can start after ~1/4 of x has landed ----
        nc.sync.dma_start(wks[:], wk_d.rearrange("(ko p) m -> p ko m", p=P))
        nc.sync.dma_start(wqs[:], wq_d.rearrange("(ko p) m -> p ko m", p=P))
        xt_r = xt_d.rearrange("(ko p) s -> p ko s", p=P)
        for n in range(NQ):
            nc.sync.dma_start(xs[:, :, n * QW:(n + 1) * QW],
                              xt_r[:, :, n * QW:(n + 1) * QW])
        nc.sync.dma_start(wvs[:], wv_d.rearrange("(ko p) m -> p ko m", p=P))
        nc.sync.dma_start(wo_sb[:], wo_d.rearrange("(c p) e -> p c e", p=P))

        # ones columns of V_aug (col HD of each VW-wide head block): bf16 1.0
        va_h = va[:].rearrange("p s (h c) -> p s h c", c=VW)
        for h in range(HG):
            nc.vector.memset(
                va_h[:, :, h, HD:HD + 1].bitcast(mybir.dt.uint16), 0x3F80)

        # ---- emission helpers ----
        def proj_tile(w_sb, dst, m, n):
            """QT/KT tile [128 x 512]: full K=128 contraction, single bank."""
            g = ps_g.tile([P, QW], F32, tag="g")
            for ko in range(KO):
                nc.tensor.matmul(g[:], w_sb[:, ko, m * P:(m + 1) * P],
                                 xs[:, ko, n * QW:(n + 1) * QW],
                                 start=(ko == 0), stop=(ko == KO - 1))
            nc.vector.tensor_copy(dst[:, m, n * QW:(n + 1) * QW], g[:])

        def v_tile(sc):
            """V natural tile for s-chunk sc: [128 x 256] into va."""
            g = ps_g.tile([P, QW], F32, tag="g")
            for ko in range(KO):
                nc.tensor.matmul(g[:, :GC], xs[:, ko, sc * P:(sc + 1) * P],
                                 wvs[:, ko, :],
                                 start=(ko == 0), stop=(ko == KO - 1))
            nc.vector.tensor_copy(
                va_h[:, sc, :, 0:HD],
                g[:, :GC].rearrange("p (h c) -> p h c", c=HD))

        ot_ref = [None]

        def ph3_mm(n, m):
            if m == 0:
                ot_ref[0] = ot_pool.tile([P, KO, QW], F32, tag="ot",
                                         name=f"ot{n}")
            g = ps_g.tile([P, QW], F32, tag="g")
            for c in range(2):
                nc.tensor.matmul(g[:], wo_sb[:, c, m * P:(m + 1) * P],
                                 ct[:, c, n * QW:(n + 1) * QW],
                                 start=(c == 0), stop=(c == 1))
            nc.vector.tensor_copy(ot_ref[0][:, m, :], g[:])

        def ph3_dma(n):
            nc.sync.dma_start(
                out_d.rearrange("(m p) q -> p m q", p=P)[:, :, n * QW:(n + 1) * QW],
                ot_ref[0][:])

        # ---- phase 1 head: KT fully (scores need all k), QT for n=0 ----
        for n in range(NQ):
            for m in range(2):
                proj_tile(wks, kt, m, n)
        for m in range(2):
            proj_tile(wqs, qt, m, 0)

        # filler queue: work the PE can chew on while the Scalar engine
        # (exp) paces phase 2. V tiles are NOT here: they are force-emitted
        # during (n0, hp0) so va[kc] always precedes ctx[kc].
        fillers = []
        for n in range(1, NQ):
            for m in range(2):
                fillers.append(lambda m=m, n=n: proj_tile(wqs, qt, m, n))

        # ---- phase 2 (+ interleaved phase 3 per finished n) ----
        for n in range(NQ):
            nsl = slice(n * QW, (n + 1) * QW)
            for hp in range(2):
                c = ps_c.tile([P, 2, QW], F32, tag="c", name=f"c{n}{hp}")
                exs = {}

                def ctx_mm(kc):
                    ex = exs.pop(kc)
                    for e in range(2):
                        h = 2 * hp + e
                        nc.tensor.matmul(
                            c[0:VW, e, :],
                            va[:, kc, h * VW:(h + 1) * VW],
                            ex[:, e, :],
                            start=(kc == 0), stop=(kc == NKC - 1))

                for kc in range(NKC):
                    sp = ps_s.tile([P, 2, QW], F32, tag="s")
                    for e in range(2):
                        nc.tensor.matmul(
                            sp[:, e, :],
                            kt[e * 64:e * 64 + 64, hp, kc * P:(kc + 1) * P],
                            qt[e * 64:e * 64 + 64, hp, nsl],
                            start=True, stop=True)
                    ex = ex_pool.tile([P, 2, QW], BF16, tag="ex")
                    nc.scalar.activation(
                        ex[:].rearrange("p a b -> p (a b)"),
                        sp[:].rearrange("p a b -> p (a b)"),
                        mybir.ActivationFunctionType.Exp,
                        scale=0.125)
                    exs[kc] = ex
                    if dbg is not None and n == 0 and hp == 0 and kc == 0:
                        nc.sync.dma_start(dbg["dbg_ex"][:], ex[:])
                    if n == 0 and hp == 0:
                        v_tile(kc)          # forced: va[kc] before ctx[kc]
                    elif fillers and kc % 5 == 4:
                        fillers.pop(0)()
                    if kc >= 2:
                        ctx_mm(kc - 2)
                ctx_mm(NKC - 2)
                ctx_mm(NKC - 1)

                # normalize: ctxT = ctx_aug[0:64] * (1 / ctx_aug[64])
                dn = dn_pool.tile([P, 4, QW], F32, tag="dn")
                bc = bc_pool.tile([P, 2, QW], F32, tag="bc")
                for e in range(2):
                    # denom row: PSUM partition 64 -> SBUF partition 0
                    nc.vector.tensor_copy(dn[0:1, e, :], c[64:65, e, :])
                    nc.gpsimd.partition_broadcast(
                        dn[0:64, 2 + e, :], dn[0:1, e, :], channels=64)
                    nc.vector.reciprocal_approx_fast(
                        bc[0:64, e, :], dn[0:64, 2 + e, :])
                    nc.vector.tensor_tensor(
                        ct[e * 64:(e + 1) * 64, hp, nsl],
                        c[0:64, e, :], bc[0:64, e, :], mult)
                if dbg is not None and n == 0 and hp == 0:
                    nc.sync.dma_start(dbg["dbg_dn"][:], dn[:])
                    nc.sync.dma_start(dbg["dbg_bc"][:], bc[:])

            # phase 3 for this finished q-block rides the filler queue
            for m in range(KO):
                fillers.append(lambda n=n, m=m: ph3_mm(n, m))
            fillers.append(lambda n=n: ph3_dma(n))

        while fillers:
            fillers.pop(0)()

        if dbg is not None:
            nc.sync.dma_start(dbg["dbg_qt"][:], qt[:])
            nc.sync.dma_start(dbg["dbg_kt"][:], kt[:])
            nc.sync.dma_start(dbg["dbg_va"][:], va[:])
            nc.sync.dma_start(dbg["dbg_ct"][:], ct[:])


def _in_maps(x, wq_f, wk_f, wv_f, wo_f):
    maps = []
    for core in range(8):
        b, g = core // 4, core % 4
        cols = slice(g * GC, (g + 1) * GC)
        maps.append({
            "xt": np.ascontiguousarray(x[b].T).astype(NP_BF16),
            "wq": np.ascontiguousarray(wq_f[:, cols]).astype(NP_BF16),
            "wk": np.ascontiguousarray(wk_f[:, cols]).astype(NP_BF16),
            "wv": np.ascontiguousarray(wv_f[:, cols]).astype(NP_BF16),
            "wo": np.ascontiguousarray(wo_f[cols, :]).astype(NP_BF16),
        })
    return maps


def _prep(x, Wq, Wk, Wv, Wo, q_scale, k_scale, v_scale, o_scale):
    x = np.asarray(x, dtype=np.float32)
    wq_f = (np.asarray(Wq).T * np.asarray(q_scale).reshape(1, -1)).astype(np.float32)
    wk_f = (np.asarray(Wk).T * np.asarray(k_scale).reshape(1, -1)).astype(np.float32)
    wv_f = (np.asarray(Wv).T * np.asarray(v_scale).reshape(1, -1)).astype(np.float32)
    wo_f = (np.asarray(Wo).T * np.asarray(o_scale).reshape(1, -1)).astype(np.float32)
    return x, wq_f, wk_f, wv_f, wo_f


def run_traced(x, Wq, Wk, Wv, Wo, q_scale, k_scale, v_scale, o_scale):
    """Like kernel() but with NTFF tracing; returns (out, exec_time_ns, trace_path)."""
    x, wq_f, wk_f, wv_f, wo_f = _prep(x, Wq, Wk, Wv, Wo,
                                      q_scale, k_scale, v_scale, o_scale)
    nc = _build()
    res = run_bass_kernel_spmd(nc, _in_maps(x, wq_f, wk_f, wv_f, wo_f),
                               core_ids=list(range(8)), trace=True)
    out = np.zeros((x.shape[0], S, D), dtype=np.float32)
    for core in range(8):
        out[core // 4] += np.asarray(res.results[core]["out_t"],
                                     dtype=np.float32).T
    trace_path = None
    if res.instructions_and_trace is not None:
        trace_path = res.instructions_and_trace[1]
    return out, res.exec_time_ns, trace_path


def kernel(x, Wq, Wk, Wv, Wo, q_scale, k_scale, v_scale, o_scale):
    B = x.shape[0]
    x, wq_f, wk_f, wv_f, wo_f = _prep(x, Wq, Wk, Wv, Wo,
                                      q_scale, k_scale, v_scale, o_scale)
    nc = _build()
    res = run_bass_kernel_spmd(nc, _in_maps(x, wq_f, wk_f, wv_f, wo_f),
                               core_ids=list(range(8)))
    out = np.zeros((B, S, D), dtype=np.float32)
    for core in range(8):
        out[core // 4] += np.asarray(res.results[core]["out_t"],
                                     dtype=np.float32).T
    return out


# revision 18
# speedup vs baseline: 1.8830x; 1.1690x over previous
"""Multi-head self-attention (B=2, S=2048, D=1024, H=16) on 8 Trainium2 NeuronCores.

Sharding: batch x head-group. Core c = b*4 + g handles batch b and heads 4g..4g+3
(Megatron-style TP: Wq/Wk/Wv column-sharded, Wo row-sharded; partial outputs
summed on the host).

Per-core kernel layout ("T-layout": sequence on the free dim everywhere),
all matmul operands bf16, PSUM accumulation fp32:
  inputs (host-prepared):  xt [1024, 2048] = x[b].T;  wq/wk/wv [1024, 256]
  (scale-folded, transposed);  wo [256, 1024] (scale-folded, transposed)
  QT/KT = (w.T @ xt) [256, 2048]        d' on partitions, heads pair-stacked
  V     = (xt.T @ wv) [2048, 260]       natural layout + ones column per head
  scoresT[k, q] = KT_h-slices.T @ QT_h  per head, k on partitions (row-tiled
                                        T0/T8 pair: both heads of a pair run
                                        concurrently on the PE)
  expT = exp(scoresT / 8)               (no max subtraction: |scores| <~ 2)
  ctxT_aug[d+1, q] = [V_h | 1].T @ expT K=128 accumulation in one PSUM bank;
                                        row 64 = softmax denominator
  ctxT = ctxT_aug[0:64] * (1/denom)     recip on DVE, denom row broadcast via
                                        gpsimd partition_broadcast
  outT_partial = wo.T @ ctxT [1024, 2048]
Host: out[b] = sum_g outT[b, g].T

Pipeline structure: phase-2 score PSUM double-buffered so the Exp ACTIVATEs
(the critical path, ~128 x [128,1024]) stream back-to-back on the Scalar
engine while the PE interleaves scores/ctx with "filler" work (V projection,
remaining QT tiles, per-n output projection) to stay HAM-warm.
"""
import sys

sys.path.insert(0, "/opt/trn_rl_repo")

import numpy as np
import ml_dtypes

import concourse.bass as bass
import concourse.tile as tile
from concourse import bacc, mybir
from concourse.bass_utils import run_bass_kernel_spmd

F32 = mybir.dt.float32
BF16 = mybir.dt.bfloat16
NP_BF16 = ml_dtypes.bfloat16

S = 2048          # sequence length per batch
D = 1024          # embedding dim
HG = 4            # heads per core
HD = 64           # head dim
GC = HG * HD      # group cols = 256
P = 128
NQ = 4            # q chunks of 512
QW = 512          # q chunk width
NKC = 16          # key-position chunks of 128
KO = 8            # contraction chunks of 128 over D
VW = HD + 1       # V columns per head incl. ones column

_NC_CACHE = {}
DEBUG_DUMPS = False


def _build():
    if "nc" in _NC_CACHE:
        return _NC_CACHE["nc"]
    nc = bacc.Bacc(trn_type="TRN2", target_bir_lowering=False, debug=False)
    xt_d = nc.dram_tensor("xt", [D, S], BF16, kind="ExternalInput")
    wq_d = nc.dram_tensor("wq", [D, GC], BF16, kind="ExternalInput")
    wk_d = nc.dram_tensor("wk", [D, GC], BF16, kind="ExternalInput")
    wv_d = nc.dram_tensor("wv", [D, GC], BF16, kind="ExternalInput")
    wo_d = nc.dram_tensor("wo", [GC, D], BF16, kind="ExternalInput")
    out_d = nc.dram_tensor("out_t", [D, S], F32, kind="ExternalOutput")
    dbg = None
    if DEBUG_DUMPS:
        dbg = {
            "dbg_qt": nc.dram_tensor("dbg_qt", [P, 2, S], BF16,
                                     kind="ExternalOutput"),
            "dbg_kt": nc.dram_tensor("dbg_kt", [P, 2, S], BF16,
                                     kind="ExternalOutput"),
            "dbg_va": nc.dram_tensor("dbg_va", [P, NKC, HG * VW], BF16,
                                     kind="ExternalOutput"),
            "dbg_ct": nc.dram_tensor("dbg_ct", [P, 2, S], BF16,
                                     kind="ExternalOutput"),
            "dbg_dn": nc.dram_tensor("dbg_dn", [P, 4, QW], F32,
                                     kind="ExternalOutput"),
            "dbg_bc": nc.dram_tensor("dbg_bc", [P, 2, QW], F32,
                                     kind="ExternalOutput"),
            "dbg_ex": nc.dram_tensor("dbg_ex", [P, 2, QW], BF16,
                                     kind="ExternalOutput"),
        }
    with tile.TileContext(nc) as tc:
        _emit(nc, tc, xt_d, wq_d, wk_d, wv_d, wo_d, out_d, dbg)
    nc.compile()
    _NC_CACHE["nc"] = nc
    return nc


def _emit(nc, tc, xt_d, wq_d, wk_d, wv_d, wo_d, out_d, dbg=None):
    mult = mybir.AluOpType.mult
    with tc.tile_pool(name="big", bufs=1) as big, \
         tc.tile_pool(name="ex", bufs=8) as ex_pool, \
         tc.tile_pool(name="dn", bufs=2) as dn_pool, \
         tc.tile_pool(name="bcn", bufs=2) as bc_pool, \
         tc.tile_pool(name="ot", bufs=2) as ot_pool, \
         tc.tile_pool(name="ps_s", bufs=3, space="PSUM") as ps_s, \
         tc.tile_pool(name="ps_c", bufs=1, space="PSUM") as ps_c:

        # ---- persistent SBUF tensors ----
        xs = big.tile([P, KO, S], BF16)          # x.T  [d_in(128) x ko x s]
        wqs = big.tile([P, KO, GC], BF16)
        wks = big.tile([P, KO, GC], BF16)
        wvs = big.tile([P, KO, GC], BF16)
        wo_sb = big.tile([P, 2, D], BF16)        # [d'(128) x chunk x e]
        qt = big.tile([P, 2, S], BF16)           # head h at parts (h%2)*64, chunk h//2
        kt = big.tile([P, 2, S], BF16)
        va = big.tile([P, NKC, HG * VW], BF16)   # V natural + ones col per head
        ct = big.tile([P, 2, S], BF16)           # normalized ctxT, same layout as qt

        # ---- input DMAs: wq + x-block0 first so QT(m0,n0) starts ~4us in ----
        xt_r = xt_d.rearrange("(ko p) s -> p ko s", p=P)
        nc.sync.dma_start(wqs[:], wq_d.rearrange("(ko p) m -> p ko m", p=P))
        nc.sync.dma_start(xs[:, :, 0:QW], xt_r[:, :, 0:QW])
        nc.sync.dma_start(wks[:], wk_d.rearrange("(ko p) m -> p ko m", p=P))
        nc.sync.dma_start(wvs[:], wv_d.rearrange("(ko p) m -> p ko m", p=P))
        for n in range(1, NQ):
            nc.sync.dma_start(xs[:, :, n * QW:(n + 1) * QW],
                              xt_r[:, :, n * QW:(n + 1) * QW])
        nc.sync.dma_start(wo_sb[:], wo_d.rearrange("(c p) e -> p c e", p=P))

        # ones columns of V_aug (col HD of each VW-wide head block): bf16 1.0
        va_h = va[:].rearrange("p s (h c) -> p s h c", c=VW)
        for h in range(HG):
            nc.vector.memset(
                va_h[:, :, h, HD:HD + 1].bitcast(mybir.dt.uint16), 0x3F80)

        # ---- emission helpers (all big PSUM from the shared ps_s ring) ----
        def g_tile():
            g = ps_s.tile([P, 2, QW], F32, tag="s", name="g")
            return g

        def proj_tile(w_sb, dst, m, n):
            """QT/KT tile [128 x 512]: full K=128 contraction, single bank."""
            g = g_tile()
            for ko in range(KO):
                nc.tensor.matmul(g[:, 0, :], w_sb[:, ko, m * P:(m + 1) * P],
                                 xs[:, ko, n * QW:(n + 1) * QW],
                                 start=(ko == 0), stop=(ko == KO - 1))
            nc.vector.tensor_copy(dst[:, m, n * QW:(n + 1) * QW], g[:, 0, :])

        def v_tile(sc):
            """V natural tile for s-chunk sc: [128 x 256] into va."""
            g = g_tile()
            for ko in range(KO):
                nc.tensor.matmul(g[:, 0, :GC], xs[:, ko, sc * P:(sc + 1) * P],
                                 wvs[:, ko, :],
                                 start=(ko == 0), stop=(ko == KO - 1))
            nc.vector.tensor_copy(
                va_h[:, sc, :, 0:HD],
                g[:, 0, :GC].rearrange("p (h c) -> p h c", c=HD))

        ot_ref = [None]
        out_r = out_d.rearrange("(m p) q -> p m q", p=P)

        def ph3_mm(n, m):
            if m == 0:
                ot_ref[0] = ot_pool.tile([P, KO, QW], F32, tag="ot",
                                         name=f"ot{n}")
            g = g_tile()
            for c in range(2):
                nc.tensor.matmul(g[:, 0, :], wo_sb[:, c, m * P:(m + 1) * P],
                                 ct[:, c, n * QW:(n + 1) * QW],
                                 start=(c == 0), stop=(c == 1))
            nc.vector.tensor_copy(ot_ref[0][:, m, :], g[:, 0, :])
            nc.sync.dma_start(
                out_r[:, m, n * QW:(n + 1) * QW], ot_ref[0][:, m, :])

        # ---- fused phase 1+2+3: one global score stream (blocks b = n*2+hp,
        # 16 kc each), ctx stream lagging OFF behind so the next block's
        # scores always cover the normalize latency; KT/QT/V force-scheduled
        # into block 0; phase 3 rides the filler queue ----
        OFF = 6
        fillers = []
        cblocks = {}
        exs = {}

        def normalize(c, n, hp):
            nsl = slice(n * QW, (n + 1) * QW)
            dn = dn_pool.tile([P, 4, QW], F32, tag="dn")
            bc = bc_pool.tile([P, 2, QW], F32, tag="bc")
            for e in range(2):
                # denom row: PSUM partition 64 -> SBUF partition 0
                nc.vector.tensor_copy(dn[0:1, e, :], c[64:65, e, :])
                nc.gpsimd.partition_broadcast(
                    dn[0:64, 2 + e, :], dn[0:1, e, :], channels=64)
                nc.vector.reciprocal_approx_fast(
                    bc[0:64, e, :], dn[0:64, 2 + e, :])
                nc.vector.tensor_tensor(
                    ct[e * 64:(e + 1) * 64, hp, nsl],
                    c[0:64, e, :], bc[0:64, e, :], mult)
            if dbg is not None and n == 0 and hp == 0:
                nc.sync.dma_start(dbg["dbg_dn"][:], dn[:])
                nc.sync.dma_start(dbg["dbg_bc"][:], bc[:])

        def extra(b, kc):
            """Forced pre-work + filler pops, scheduled per stream slot."""
            if b == 0:
                # interleave all of QT(n0)/KT/V into block 0 so the Scalar
                # engine starts on exp ~6us into the kernel
                if kc == 0:
                    proj_tile(wqs, qt, 0, 0)
                    proj_tile(wks, kt, 0, 0)
                elif kc == 1:
                    proj_tile(wqs, qt, 1, 0)
                elif kc % 4 == 0:
                    proj_tile(wks, kt, 0, kc // 4)
                elif kc % 4 == 2:
                    proj_tile(wks, kt, 1, kc // 4)
                v_tile(kc)              # va[kc] always precedes ctx[kc]
                return
            n, hp = divmod(b, 2)
            if hp == 1 and n + 1 < NQ and kc in (2, 4):
                proj_tile(wqs, qt, kc // 2 - 1, n + 1)   # m=0 at kc2, m=1 at kc4
            elif fillers and kc % 2 == 1:
                fillers.pop(0)()

        total = 2 * NQ * NKC
        for gpos in range(total + OFF):
            if gpos < total:
                b, kc = divmod(gpos, NKC)
                n, hp = divmod(b, 2)
                nsl = slice(n * QW, (n + 1) * QW)
                extra(b, kc)
                # scores(b, kc): scores(b,kc) needs KT chunk hp, QT(n) chunk hp
                sp = ps_s.tile([P, 2, QW], F32, tag="s")
                for e in range(2):
                    nc.tensor.matmul(
                        sp[:, e, :],
                        kt[e * 64:e * 64 + 64, hp, kc * P:(kc + 1) * P],
                        qt[e * 64:e * 64 + 64, hp, nsl],
                        start=True, stop=True)
                ex = ex_pool.tile([P, 2, QW], BF16, tag="ex")
                nc.scalar.activation(
                    ex[:].rearrange("p a b -> p (a b)"),
                    sp[:].rearrange("p a b -> p (a b)"),
                    mybir.ActivationFunctionType.Exp,
                    scale=0.125)
                exs[gpos] = ex
                if dbg is not None and gpos == 0:
                    nc.sync.dma_start(dbg["dbg_ex"][:], ex[:])
            if gpos >= OFF:
                g2 = gpos - OFF
                b2, kc2 = divmod(g2, NKC)
                n2, hp2 = divmod(b2, 2)
                if kc2 == 0:
                    cblocks[b2] = ps_c.tile([P, 2, QW], F32, tag="c",
                                            name=f"c{b2}")
                c = cblocks[b2]
                ex2 = exs.pop(g2)
                for e in range(2):
                    h = 2 * hp2 + e
                    nc.tensor.matmul(
                        c[0:VW, e, :],
                        va[:, kc2, h * VW:(h + 1) * VW],
                        ex2[:, e, :],
                        start=(kc2 == 0), stop=(kc2 == NKC - 1))
                if kc2 == NKC - 1:
                    normalize(cblocks.pop(b2), n2, hp2)
                    if hp2 == 1:
                        for m in range(KO):
                            fillers.append(lambda n=n2, m=m: ph3_mm(n, m))

        while fillers:
            fillers.pop(0)()

        if dbg is not None:
            nc.sync.dma_start(dbg["dbg_qt"][:], qt[:])
            nc.sync.dma_start(dbg["dbg_kt"][:], kt[:])
            nc.sync.dma_start(dbg["dbg_va"][:], va[:])
            nc.sync.dma_start(dbg["dbg_ct"][:], ct[:])


def _in_maps(x, wq_f, wk_f, wv_f, wo_f):
    maps = []
    for core in range(8):
        b, g = core // 4, core % 4
        cols = slice(g * GC, (g + 1) * GC)
        maps.append({
            "xt": np.ascontiguousarray(x[b].T).astype(NP_BF16),
            "wq": np.ascontiguousarray(wq_f[:, cols]).astype(NP_BF16),
            "wk": np.ascontiguousarray(wk_f[:, cols]).astype(NP_BF16),
            "wv": np.ascontiguousarray(wv_f[:, cols]).astype(NP_BF16),
            "wo": np.ascontiguousarray(wo_f[cols, :]).astype(NP_BF16),
        })
    return maps


def _prep(x, Wq, Wk, Wv, Wo, q_scale, k_scale, v_scale, o_scale):
    x = np.asarray(x, dtype=np.float32)
    wq_f = (np.asarray(Wq).T * np.asarray(q_scale).reshape(1, -1)).astype(np.float32)
    wk_f = (np.asarray(Wk).T * np.asarray(k_scale).reshape(1, -1)).astype(np.float32)
    wv_f = (np.asarray(Wv).T * np.asarray(v_scale).reshape(1, -1)).astype(np.float32)
    wo_f = (np.asarray(Wo).T * np.asarray(o_scale).reshape(1, -1)).astype(np.float32)
    return x, wq_f, wk_f, wv_f, wo_f


def run_traced(x, Wq, Wk, Wv, Wo, q_scale, k_scale, v_scale, o_scale):
    """Like kernel() but with NTFF tracing; returns (out, exec_time_ns, trace_path)."""
    x, wq_f, wk_f, wv_f, wo_f = _prep(x, Wq, Wk, Wv, Wo,
                                      q_scale, k_scale, v_scale, o_scale)
    nc = _build()
    res = run_bass_kernel_spmd(nc, _in_maps(x, wq_f, wk_f, wv_f, wo_f),
                               core_ids=list(range(8)), trace=True)
    out = np.zeros((x.shape[0], S, D), dtype=np.float32)
    for core in range(8):
        out[core // 4] += np.asarray(res.results[core]["out_t"],
                                     dtype=np.float32).T
    trace_path = None
    if res.instructions_and_trace is not None:
        trace_path = res.instructions_and_trace[1]
    return out, res.exec_time_ns, trace_path


def kernel(x, Wq, Wk, Wv, Wo, q_scale, k_scale, v_scale, o_scale):
    B = x.shape[0]
    x, wq_f, wk_f, wv_f, wo_f = _prep(x, Wq, Wk, Wv, Wo,
                                      q_scale, k_scale, v_scale, o_scale)
    nc = _build()
    res = run_bass_kernel_spmd(nc, _in_maps(x, wq_f, wk_f, wv_f, wo_f),
                               core_ids=list(range(8)))
    out = np.zeros((B, S, D), dtype=np.float32)
    for core in range(8):
        out[core // 4] += np.asarray(res.results[core]["out_t"],
                                     dtype=np.float32).T
    return out


# revision 21
# speedup vs baseline: 2.0132x; 1.0691x over previous
"""Multi-head self-attention (B=2, S=2048, D=1024, H=16) on 8 Trainium2 NeuronCores.

Sharding: batch x head-group. Core c = b*4 + g handles batch b and heads 4g..4g+3
(Megatron-style TP: Wq/Wk/Wv column-sharded, Wo row-sharded; partial outputs
summed on the host).

Per-core kernel layout ("T-layout": sequence on the free dim everywhere),
all matmul operands bf16, PSUM accumulation fp32:
  inputs (host-prepared):  xt [1024, 2048] = x[b].T;  wq/wk/wv [1024, 256]
  (scale-folded, transposed);  wo [256, 1024] (scale-folded, transposed)
  QT/KT = (w.T @ xt) [256, 2048]        d' on partitions, heads pair-stacked
  V     = (xt.T @ wv) [2048, 260]       natural layout + ones column per head
  scoresT[k, q] = KT_h-slices.T @ QT_h  per head, k on partitions (row-tiled
                                        T0/T8 pair: both heads of a pair run
                                        concurrently on the PE)
  expT = exp(scoresT / 8)               (no max subtraction: |scores| <~ 2)
  ctxT_aug[d+1, q] = [V_h | 1].T @ expT K=128 accumulation in one PSUM bank;
                                        row 64 = softmax denominator
  ctxT = ctxT_aug[0:64] * (1/denom)     recip on DVE, denom row broadcast via
                                        gpsimd partition_broadcast
  outT_partial = wo.T @ ctxT [1024, 2048]
Host: out[b] = sum_g outT[b, g].T

Pipeline structure: phase-2 score PSUM double-buffered so the Exp ACTIVATEs
(the critical path, ~128 x [128,1024]) stream back-to-back on the Scalar
engine while the PE interleaves scores/ctx with "filler" work (V projection,
remaining QT tiles, per-n output projection) to stay HAM-warm.
"""
import sys

sys.path.insert(0, "/opt/trn_rl_repo")

import numpy as np
import ml_dtypes

import concourse.bass as bass
import concourse.tile as tile
from concourse import bacc, mybir
from concourse.bass_utils import run_bass_kernel_spmd

F32 = mybir.dt.float32
BF16 = mybir.dt.bfloat16
NP_BF16 = ml_dtypes.bfloat16

S = 2048          # sequence length per batch
D = 1024          # embedding dim
HG = 4            # heads per core
HD = 64           # head dim
GC = HG * HD      # group cols = 256
P = 128
NQ = 4            # q chunks of 512
QW = 512          # q chunk width
NKC = 16          # key-position chunks of 128
KO = 8            # contraction chunks of 128 over D
VW = HD + 1       # V columns per head incl. ones column

_NC_CACHE = {}
DEBUG_DUMPS = False


def _build():
    if "nc" in _NC_CACHE:
        return _NC_CACHE["nc"]
    nc = bacc.Bacc(trn_type="TRN2", target_bir_lowering=False, debug=False)
    xt_d = nc.dram_tensor("xt", [D, S], BF16, kind="ExternalInput")
    wq_d = nc.dram_tensor("wq", [D, GC], BF16, kind="ExternalInput")
    wk_d = nc.dram_tensor("wk", [D, GC], BF16, kind="ExternalInput")
    wv_d = nc.dram_tensor("wv", [D, GC], BF16, kind="ExternalInput")
    wo_d = nc.dram_tensor("wo", [GC, D], BF16, kind="ExternalInput")
    out_d = nc.dram_tensor("out_t", [D, S], F32, kind="ExternalOutput")
    dbg = None
    if DEBUG_DUMPS:
        dbg = {
            "dbg_qt": nc.dram_tensor("dbg_qt", [P, 2, S], BF16,
                                     kind="ExternalOutput"),
            "dbg_kt": nc.dram_tensor("dbg_kt", [P, 2, S], BF16,
                                     kind="ExternalOutput"),
            "dbg_va": nc.dram_tensor("dbg_va", [P, NKC, HG * VW], BF16,
                                     kind="ExternalOutput"),
            "dbg_ct": nc.dram_tensor("dbg_ct", [P, 2, S], BF16,
                                     kind="ExternalOutput"),
            "dbg_dn": nc.dram_tensor("dbg_dn", [P, 4, QW], F32,
                                     kind="ExternalOutput"),
            "dbg_bc": nc.dram_tensor("dbg_bc", [P, 2, QW], F32,
                                     kind="ExternalOutput"),
            "dbg_ex": nc.dram_tensor("dbg_ex", [P, 2, QW], BF16,
                                     kind="ExternalOutput"),
        }
    with tile.TileContext(nc) as tc:
        _emit(nc, tc, xt_d, wq_d, wk_d, wv_d, wo_d, out_d, dbg)
    nc.compile()
    _NC_CACHE["nc"] = nc
    return nc


def _emit(nc, tc, xt_d, wq_d, wk_d, wv_d, wo_d, out_d, dbg=None):
    mult = mybir.AluOpType.mult
    with tc.tile_pool(name="big", bufs=1) as big, \
         tc.tile_pool(name="ex", bufs=10) as ex_pool, \
         tc.tile_pool(name="dn", bufs=2) as dn_pool, \
         tc.tile_pool(name="bcn", bufs=2) as bc_pool, \
         tc.tile_pool(name="ot", bufs=2) as ot_pool, \
         tc.tile_pool(name="ps_s", bufs=3, space="PSUM") as ps_s, \
         tc.tile_pool(name="ps_c", bufs=1, space="PSUM") as ps_c:

        # ---- persistent SBUF tensors ----
        xs = big.tile([P, KO, S], BF16)          # x.T  [d_in(128) x ko x s]
        wqs = big.tile([P, KO, GC], BF16)
        wks = big.tile([P, KO, GC], BF16)
        wvs = big.tile([P, KO, GC], BF16)
        wo_sb = big.tile([P, 2, D], BF16)        # [d'(128) x chunk x e]
        qt = big.tile([P, 2, S], BF16)           # head h at parts (h%2)*64, chunk h//2
        kt = big.tile([P, 2, S], BF16)
        va = big.tile([P, NKC, HG * VW], BF16)   # V natural + ones col per head
        ct = big.tile([P, 2, S], BF16)           # normalized ctxT, same layout as qt

        # ---- input DMAs: wq + x-block0 first so QT(m0,n0) starts ~4us in ----
        xt_r = xt_d.rearrange("(ko p) s -> p ko s", p=P)
        nc.sync.dma_start(wqs[:], wq_d.rearrange("(ko p) m -> p ko m", p=P))
        nc.sync.dma_start(xs[:, :, 0:QW], xt_r[:, :, 0:QW])
        nc.sync.dma_start(wks[:], wk_d.rearrange("(ko p) m -> p ko m", p=P))
        nc.sync.dma_start(wvs[:], wv_d.rearrange("(ko p) m -> p ko m", p=P))
        for n in range(1, NQ):
            nc.sync.dma_start(xs[:, :, n * QW:(n + 1) * QW],
                              xt_r[:, :, n * QW:(n + 1) * QW])
        nc.sync.dma_start(wo_sb[:], wo_d.rearrange("(c p) e -> p c e", p=P))

        # ones columns of V_aug (col HD of each VW-wide head block): bf16 1.0
        va_h = va[:].rearrange("p s (h c) -> p s h c", c=VW)
        for h in range(HG):
            nc.vector.memset(
                va_h[:, :, h, HD:HD + 1].bitcast(mybir.dt.uint16), 0x3F80)

        # ---- emission helpers (all big PSUM from the shared ps_s ring) ----
        def g_tile():
            g = ps_s.tile([P, 2, QW], F32, tag="s", name="g")
            return g

        def proj_tile(w_sb, dst, m, n):
            """QT/KT tile [128 x 512]: full K=128 contraction, single bank."""
            g = g_tile()
            for ko in range(KO):
                nc.tensor.matmul(g[:, 0, :], w_sb[:, ko, m * P:(m + 1) * P],
                                 xs[:, ko, n * QW:(n + 1) * QW],
                                 start=(ko == 0), stop=(ko == KO - 1))
            nc.vector.tensor_copy(dst[:, m, n * QW:(n + 1) * QW], g[:, 0, :])

        def v_tile(sc):
            """V natural tile for s-chunk sc: [128 x 256] into va."""
            g = g_tile()
            for ko in range(KO):
                nc.tensor.matmul(g[:, 0, :GC], xs[:, ko, sc * P:(sc + 1) * P],
                                 wvs[:, ko, :],
                                 start=(ko == 0), stop=(ko == KO - 1))
            nc.vector.tensor_copy(
                va_h[:, sc, :, 0:HD],
                g[:, 0, :GC].rearrange("p (h c) -> p h c", c=HD))

        ot_ref = [None]
        out_r = out_d.rearrange("(m p) q -> p m q", p=P)

        def ph3_mm(n, m):
            if m == 0:
                ot_ref[0] = ot_pool.tile([P, KO, QW], F32, tag="ot",
                                         name=f"ot{n}")
            g = g_tile()
            for c in range(2):
                nc.tensor.matmul(g[:, 0, :], wo_sb[:, c, m * P:(m + 1) * P],
                                 ct[:, c, n * QW:(n + 1) * QW],
                                 start=(c == 0), stop=(c == 1))
            nc.vector.tensor_copy(ot_ref[0][:, m, :], g[:, 0, :])
            nc.sync.dma_start(
                out_r[:, m, n * QW:(n + 1) * QW], ot_ref[0][:, m, :])

        # ---- fused phase 1+2+3: one global score stream (blocks b = n*2+hp,
        # 16 kc each), ctx stream lagging OFF behind so the next block's
        # scores always cover the normalize latency; KT/QT/V force-scheduled
        # into block 0; phase 3 rides the filler queue ----
        OFF = 6
        fillers = []
        cblocks = {}
        exs = {}

        def normalize(c, n, hp):
            nsl = slice(n * QW, (n + 1) * QW)
            dn = dn_pool.tile([P, 4, QW], F32, tag="dn")
            bc = bc_pool.tile([P, 2, QW], F32, tag="bc")
            for e in range(2):
                # denom row: PSUM partition 64 -> SBUF partition 0
                nc.vector.tensor_copy(dn[0:1, e, :], c[64:65, e, :])
                nc.gpsimd.partition_broadcast(
                    dn[0:64, 2 + e, :], dn[0:1, e, :], channels=64)
                nc.vector.reciprocal_approx_fast(
                    bc[0:64, e, :], dn[0:64, 2 + e, :])
                nc.vector.tensor_tensor(
                    ct[e * 64:(e + 1) * 64, hp, nsl],
                    c[0:64, e, :], bc[0:64, e, :], mult)
            if dbg is not None and n == 0 and hp == 0:
                nc.sync.dma_start(dbg["dbg_dn"][:], dn[:])
                nc.sync.dma_start(dbg["dbg_bc"][:], bc[:])

        def extra(b, kc):
            """Forced pre-work + filler pops, scheduled per stream slot."""
            n, hp = divmod(b, 2)
            if b == 0:
                # interleave QT(n0)/KT(m0)/V into block 0 so the Scalar
                # engine starts on exp ~6us into the kernel
                if kc == 0:
                    proj_tile(wqs, qt, 0, 0)
                    proj_tile(wks, kt, 0, 0)
                elif kc == 1:
                    proj_tile(wqs, qt, 1, 0)
                elif kc % 4 == 0:
                    proj_tile(wks, kt, 0, kc // 4)
                v_tile(kc)              # va[kc] always precedes ctx[kc]
                return
            if b == 1 and kc % 4 == 0:
                proj_tile(wks, kt, 1, kc // 4)     # KT(m1) just-in-time
                return
            qt_slots = (3, 5) if b == 1 else (2, 4)
            if hp == 1 and n + 1 < NQ and kc in qt_slots:
                proj_tile(wqs, qt, qt_slots.index(kc), n + 1)
            elif fillers and kc % 2 == 1:
                fillers.pop(0)()

        # block-local schedule: scores at slots 0..15, ctx catches up two per
        # slot at slots 8..15, so the next block's first ctx sits ~9 score
        # slots behind the previous normalize (covers its latency).
        CTX0 = NKC // 2
        for b in range(2 * NQ):
            n, hp = divmod(b, 2)
            nsl = slice(n * QW, (n + 1) * QW)
            c = ps_c.tile([P, 2, QW], F32, tag="c", name=f"c{b}")

            def ctx_mm(kc2):
                ex2 = exs.pop(kc2)
                for e in range(2):
                    h = 2 * hp + e
                    nc.tensor.matmul(
                        c[0:VW, e, :],
                        va[:, kc2, h * VW:(h + 1) * VW],
                        ex2[:, e, :],
                        start=(kc2 == 0), stop=(kc2 == NKC - 1))

            for kc in range(NKC):
                extra(b, kc)
                sp = ps_s.tile([P, 2, QW], F32, tag="s")
                for e in range(2):
                    nc.tensor.matmul(
                        sp[:, e, :],
                        kt[e * 64:e * 64 + 64, hp, kc * P:(kc + 1) * P],
                        qt[e * 64:e * 64 + 64, hp, nsl],
                        start=True, stop=True)
                ex = ex_pool.tile([P, 2, QW], BF16, tag="ex")
                nc.scalar.activation(
                    ex[:].rearrange("p a b -> p (a b)"),
                    sp[:].rearrange("p a b -> p (a b)"),
                    mybir.ActivationFunctionType.Exp,
                    scale=0.125)
                exs[kc] = ex
                if dbg is not None and b == 0 and kc == 0:
                    nc.sync.dma_start(dbg["dbg_ex"][:], ex[:])
                if kc >= CTX0:
                    ctx_mm(2 * (kc - CTX0))
                    ctx_mm(2 * (kc - CTX0) + 1)
            normalize(c, n, hp)
            if hp == 1:
                for m in range(KO):
                    fillers.append(lambda n=n, m=m: ph3_mm(n, m))

        while fillers:
            fillers.pop(0)()

        if dbg is not None:
            nc.sync.dma_start(dbg["dbg_qt"][:], qt[:])
            nc.sync.dma_start(dbg["dbg_kt"][:], kt[:])
            nc.sync.dma_start(dbg["dbg_va"][:], va[:])
            nc.sync.dma_start(dbg["dbg_ct"][:], ct[:])


def _in_maps(x, wq_f, wk_f, wv_f, wo_f):
    maps = []
    for core in range(8):
        b, g = core // 4, core % 4
        cols = slice(g * GC, (g + 1) * GC)
        maps.append({
            "xt": np.ascontiguousarray(x[b].T).astype(NP_BF16),
            "wq": np.ascontiguousarray(wq_f[:, cols]).astype(NP_BF16),
            "wk": np.ascontiguousarray(wk_f[:, cols]).astype(NP_BF16),
            "wv": np.ascontiguousarray(wv_f[:, cols]).astype(NP_BF16),
            "wo": np.ascontiguousarray(wo_f[cols, :]).astype(NP_BF16),
        })
    return maps


def _prep(x, Wq, Wk, Wv, Wo, q_scale, k_scale, v_scale, o_scale):
    x = np.asarray(x, dtype=np.float32)
    wq_f = (np.asarray(Wq).T * np.asarray(q_scale).reshape(1, -1)).astype(np.float32)
    wk_f = (np.asarray(Wk).T * np.asarray(k_scale).reshape(1, -1)).astype(np.float32)
    wv_f = (np.asarray(Wv).T * np.asarray(v_scale).reshape(1, -1)).astype(np.float32)
    wo_f = (np.asarray(Wo).T * np.asarray(o_scale).reshape(1, -1)).astype(np.float32)
    return x, wq_f, wk_f, wv_f, wo_f


def run_traced(x, Wq, Wk, Wv, Wo, q_scale, k_scale, v_scale, o_scale):
    """Like kernel() but with NTFF tracing; returns (out, exec_time_ns, trace_path)."""
    x, wq_f, wk_f, wv_f, wo_f = _prep(x, Wq, Wk, Wv, Wo,
                                      q_scale, k_scale, v_scale, o_scale)
    nc = _build()
    res = run_bass_kernel_spmd(nc, _in_maps(x, wq_f, wk_f, wv_f, wo_f),
                               core_ids=list(range(8)), trace=True)
    out = np.zeros((x.shape[0], S, D), dtype=np.float32)
    for core in range(8):
        out[core // 4] += np.asarray(res.results[core]["out_t"],
                                     dtype=np.float32).T
    trace_path = None
    if res.instructions_and_trace is not None:
        trace_path = res.instructions_and_trace[1]
    return out, res.exec_time_ns, trace_path


def kernel(x, Wq, Wk, Wv, Wo, q_scale, k_scale, v_scale, o_scale):
    B = x.shape[0]
    x, wq_f, wk_f, wv_f, wo_f = _prep(x, Wq, Wk, Wv, Wo,
                                      q_scale, k_scale, v_scale, o_scale)
    nc = _build()
    res = run_bass_kernel_spmd(nc, _in_maps(x, wq_f, wk_f, wv_f, wo_f),
                               core_ids=list(range(8)))
    out = np.zeros((B, S, D), dtype=np.float32)
    for core in range(8):
        out[core // 4] += np.asarray(res.results[core]["out_t"],
                                     dtype=np.float32).T
    return out
